# revision 2
# baseline (speedup 1.0000x reference)
"""Trainium2 Bass kernel v2 for the skewed diagonal BiLSTM.

Data-parallel over batch (2 per core). Key points vs the v1 baseline:
  - K=128 i2s contractions: x kept as [128ch, b, h, w]; one matmul per
    (m-tile, b, psum-bank) instead of four K=64 passes.
  - s2s taps as two K=64 window matmuls on lh directly (PSUM h/w-window
    trick), no shifted state copies anywhere.
  - b lives in the free dim for matmul/sigmoid tiles; cell state (lc) is
    b-packed in the partition dim so tanh is one [128, 1024] op per
    direction; per-b gate column permutations keep every DVE op's inputs
    partition-aligned (walrus checkSBSameStartPartition).
  - Software-pipelined emission: each direction-step first finishes the
    previous step's cell tail (tanh, lh), then i2s matmuls (psum WARs long
    satisfied), then taps (single fresh lh wait), sigmoids, cell head.
    Directions alternate; b0's chain runs ahead of b1's.
  - All weights arrive in 3 packed DMAs; epilogue reuses the tap window
    trick for shift_down(rh) so no extra copies.
"""

import numpy as np
import ml_dtypes

B, F, H, W = 16, 64, 32, 32
C2 = 2 * F
NCORES = 8
BPC = B // NCORES  # 2

_CACHE = {}


def _get_nc(n_steps=H, reps=1):
    key = ("nc2", n_steps, reps)
    if key in _CACHE:
        return _CACHE[key]
    import sys
    if "/opt/trn_rl_repo" not in sys.path:
        sys.path.insert(0, "/opt/trn_rl_repo")
    from contextlib import ExitStack
    import concourse.mybir as mybir
    import concourse.tile as tile
    from concourse import bacc

    dt = mybir.dt
    AF = mybir.ActivationFunctionType
    OP = mybir.AluOpType

    nc = bacc.Bacc("TRN2", num_devices=NCORES)

    xd = nc.dram_tensor("x", [BPC, C2, H, W], dt.float32, kind="ExternalInput")
    # w128: [128, 2*256 wx(b-major) + 128 wsk]; w64: [64, 8*256 taps + 128 wsk64]
    w128d = nc.dram_tensor("w128", [C2, 640], dt.bfloat16, kind="ExternalInput")
    w64d = nc.dram_tensor("w64", [64, 2176], dt.bfloat16, kind="ExternalInput")
    biasd = nc.dram_tensor("biases", [C2, 9], dt.float32, kind="ExternalInput")
    yd = nc.dram_tensor("y", [BPC, C2, H, W], dt.float32, kind="ExternalOutput")

    DIRS = ("L", "R")

    with tile.TileContext(nc) as tc, ExitStack() as ctx:
        const = ctx.enter_context(tc.tile_pool(name="const", bufs=1))
        psum = ctx.enter_context(tc.tile_pool(name="psum", bufs=1, space="PSUM"))
        sigp = ctx.enter_context(tc.tile_pool(name="sig", bufs=1))
        tmp = ctx.enter_context(tc.tile_pool(name="tmp", bufs=1))
        outp = ctx.enter_context(tc.tile_pool(name="outp", bufs=1))

        w128 = const.tile([C2, 640], dt.bfloat16, name="w128_t")
        nc.sync.dma_start(out=w128[:], in_=w128d.ap())
        w64 = const.tile([64, 2176], dt.bfloat16, name="w64_t")
        nc.sync.dma_start(out=w64[:], in_=w64d.ap())
        bias_t = const.tile([C2, 9], dt.float32, name="bias_t")
        nc.sync.dma_start(out=bias_t[:], in_=biasd.ap())

        def wx_ap(b, m):
            return w128[:, b * 256 + m * C2: b * 256 + (m + 1) * C2]

        def tap_ap(d, k, b, m):  # k=1: same-row tap (w1), k=0: row-above (w0)
            base = ({"L": 0, "R": 1}[d] * 4 + (1 - k) * 2 + b) * 256
            return w64[:, base + m * C2: base + (m + 1) * C2]

        wsk64 = w64[:, 2048:2176]
        bias = {"L": bias_t[:, 0:4], "R": bias_t[:, 4:8]}
        bsk = bias_t[:, 8:9]

        # x: fp32 for residual, bf16 for matmul rhs; [128ch, b, h, w]
        xf = const.tile([C2, BPC, H, W], dt.float32, name="xf")
        for b in range(BPC):
            nc.sync.dma_start(out=xf[:, b], in_=xd.ap()[b])
        xb = const.tile([C2, BPC, H, W], dt.bfloat16, name="xb")
        for b in range(BPC):
            nc.vector.tensor_copy(xb[:, b], xf[:, b])

        # Persistent state tiles (stable across steps; Tile tracks WAR/RAW).
        lh = {d: const.tile([64, BPC, H, W], dt.bfloat16, name=f"lh{d}")
              for d in DIRS}
        th = {d: const.tile([C2, H, W], dt.bfloat16, name=f"th{d}") for d in DIRS}
        t1t = {d: const.tile([64, BPC, H, W], dt.bfloat16, name=f"t1{d}")
               for d in DIRS}
        t2t = {d: const.tile([64, BPC, H, W], dt.bfloat16, name=f"t2{d}")
               for d in DIRS}

        mm = nc.tensor.matmul
        rep_ctx = tc.For_i(0, reps, 1) if reps > 1 else None
        if rep_ctx is not None:
            rep_ctx.__enter__()

        lc = {"L": None, "R": None}   # lc tile of previous step, per dir
        sig = {"L": None, "R": None}  # [m] sig tiles of previous step

        def cell_tail(d, t):
            """tanh + lh for step t (emitted at the start of step t+1).
            Gate layout: b0: m0=(fg|ig), m1=(o|g); b1: m0=(ig|fg), m1=(g|o)."""
            nc.scalar.activation(th[d][:], lc[d][:], AF.Tanh)
            s1 = sig[d][1]
            for b in range(BPC):
                bs = slice(b * 64, b * 64 + 64)
                nc.vector.tensor_tensor(
                    lh[d][:, b], s1[bs, b], th[d][bs], OP.mult)

        def taps(d, ps, m, b):
            # w1 tap (same row) then w0 tap (row above), both as K=64
            # window matmuls; w-shift via psum/rhs column windows.
            ws = slice(1, 32) if d == "L" else slice(0, 31)
            rs = slice(0, 31) if d == "L" else slice(1, 32)
            for c in range(2):
                hs = slice(c * 16, c * 16 + 16)
                mm(ps[m][b][:, hs, ws], tap_ap(d, 1, b, m),
                   lh[d][:, b, hs, rs],
                   start=False, stop=False, skip_group_check=True)
                hso = slice(c * 16 + (1 if c == 0 else 0), c * 16 + 16)
                hsr = slice(max(c * 16 - 1, 0), c * 16 + 15)
                mm(ps[m][b][:, hso, ws], tap_ap(d, 0, b, m),
                   lh[d][:, b, hsr, rs],
                   start=False, stop=True, skip_group_check=True)

        for t in range(n_steps):
            for d in DIRS:
                if t > 0:
                    cell_tail(d, t - 1)

                # i2s chunks first (psum WARs long satisfied), then taps per
                # b (one fresh lh wait) so the recurrence never head-blocks
                # the i2s stream.
                ps = [[psum.tile([C2, H, W], dt.float32, tag=f"ps{m}{b}",
                                 name=f"ps_{t}_{d}_{m}{b}")
                       for b in range(BPC)] for m in range(2)]
                for m in range(2):
                    for b in range(BPC):
                        for c in range(2):
                            hs = slice(c * 16, c * 16 + 16)
                            mm(ps[m][b][:, hs, :], wx_ap(b, m),
                               xb[:, b, hs, :],
                               start=True, stop=(t == 0), skip_group_check=True)
                if t > 0:
                    for b in range(BPC):
                        for m in range(2):
                            taps(d, ps, m, b)

                # sigmoids + cell head, b-major so b0's chain runs ahead
                sg = [sigp.tile([C2, BPC, H, W], dt.bfloat16, tag=f"sg{d}{m}",
                                name=f"sg_{t}_{d}_{m}") for m in range(2)]
                lcn = tmp.tile([C2, H, W], dt.bfloat16, tag=f"lc{d}{t % 2}",
                               name=f"lc_{t}_{d}")
                for b in range(BPC):
                    bs = slice(b * 64, b * 64 + 64)
                    for m in range(2):
                        bc = 2 * b + m
                        nc.scalar.activation(sg[m][:, b], ps[m][b][:],
                                             AF.Sigmoid,
                                             bias=bias[d][:, bc:bc + 1])
                    gs = slice(64, 128) if b == 0 else slice(0, 64)
                    if t > 0:
                        nc.vector.tensor_tensor(
                            t2t[d][:, b], sg[0][bs, b], lc[d][bs], OP.mult)
                        nc.vector.tensor_tensor(
                            t1t[d][:, b], sg[0][gs, b], sg[1][gs, b], OP.mult)
                        nc.vector.tensor_tensor(
                            lcn[bs], t1t[d][:, b], t2t[d][:, b], OP.add)
                    else:
                        nc.vector.tensor_tensor(
                            lcn[bs], sg[0][gs, b], sg[1][gs, b], OP.mult)
                lc[d] = lcn
                sig[d] = sg

        for d in DIRS:
            cell_tail(d, n_steps - 1)

        # epilogue: skip = wsk@(lh_L + shift_down(lh_R)) + bsk; y = x + skip.
        # shift_down via the same psum h-window trick (row 0 gets no R term).
        yb = outp.tile([C2, BPC, H, W], dt.float32, tag="yb", name="yb")
        for b in range(BPC):
            psk = psum.tile([C2, H, W], dt.float32, tag=f"ps0{b}",
                            name=f"psk_{b}")
            for c in range(2):
                hs = slice(c * 16, c * 16 + 16)
                mm(psk[:, hs, :], wsk64, lh["L"][:, b, hs, :],
                   start=True, stop=False, skip_group_check=True)
                hso = slice(c * 16 + (1 if c == 0 else 0), c * 16 + 16)
                hsr = slice(max(c * 16 - 1, 0), c * 16 + 15)
                mm(psk[:, hso, :], wsk64, lh["R"][:, b, hsr, :],
                   start=False, stop=True, skip_group_check=True)
            nc.scalar.activation(yb[:, b], psk[:], AF.Identity, bias=bsk)
        ys = outp.tile([C2, BPC, H, W], dt.float32, tag="ys", name="ys")
        for b in range(BPC):
            nc.vector.tensor_tensor(ys[:, b], yb[:, b], xf[:, b], OP.add)
            nc.sync.dma_start(out=yd.ap()[b], in_=ys[:, b])

        if rep_ctx is not None:
            rep_ctx.__exit__(None, None, None)

    nc.finalize()
    _CACHE[key] = nc
    return nc


def _prep_weights(w_i2s, w_left, b_left, w_right, b_right, w_skip, b_skip):
    bf16 = ml_dtypes.bfloat16
    f32 = np.float32

    # per-b gate column permutations (keeps every DVE op partition-aligned):
    #   b0: m0 = (fg | ig), m1 = (o | g);  b1: m0 = (ig | fg), m1 = (g | o)
    P0 = np.r_[64:128, 128:192, 0:64, 192:256]
    P1 = np.r_[128:192, 64:128, 192:256, 0:64]

    wxT = np.asarray(w_i2s, f32).T                      # [128, 256]
    wskT = np.asarray(w_skip, f32).T                    # [64, 128]
    w128 = np.concatenate(
        [wxT[:, P0], wxT[:, P1],
         np.concatenate([wskT, wskT], axis=0)], axis=1)  # [128, 640]

    cols64 = []
    for w in (w_left, w_right):
        w = np.asarray(w, f32)
        for k in (1, 0):                                # w1 then w0
            t = w[:, :, k].T                            # [64, 256]
            cols64 += [t[:, P0], t[:, P1]]
    cols64.append(wskT)                                  # [64, 128]
    w64 = np.concatenate(cols64, axis=1)                 # [64, 2176]

    def bias4(bvec):  # [256] -> [128, 4], column = 2*b + m
        b = np.asarray(bvec, f32)
        return np.stack([b[P0[:128]], b[P0[128:]], b[P1[:128]], b[P1[128:]]],
                        axis=1)

    biases = np.concatenate(
        [bias4(b_left), bias4(b_right),
         np.asarray(b_skip, f32).reshape(C2, 1)], axis=1)  # [128, 9]

    return dict(w128=np.ascontiguousarray(w128).astype(bf16),
                w64=np.ascontiguousarray(w64).astype(bf16),
                biases=np.ascontiguousarray(biases))


def kernel(x, w_i2s, w_left, b_left, w_right, b_right, w_skip, b_skip):
    import os
    import sys
    if "/opt/trn_rl_repo" not in sys.path:
        sys.path.insert(0, "/opt/trn_rl_repo")
    from concourse.bass_utils import run_bass_kernel_spmd

    nc = _get_nc()
    wdict = _prep_weights(w_i2s, w_left, b_left, w_right, b_right,
                          w_skip, b_skip)
    xf = np.ascontiguousarray(np.asarray(x, np.float32))
    in_maps = [dict(wdict, x=np.ascontiguousarray(xf[i * BPC:(i + 1) * BPC]))
               for i in range(NCORES)]
    kwargs = {}
    if os.environ.get("BILSTM_TRACE"):
        kwargs = dict(trace=True, trace_cores=[0])
    res = run_bass_kernel_spmd(nc, in_maps, core_ids=list(range(NCORES)), **kwargs)
    _CACHE["last_results"] = res
    return np.concatenate([r["y"] for r in res.results], axis=0)
